# revision 13
# baseline (speedup 1.0000x reference)
"""Trainium2 Bass kernel for nn_EnokeeEncoder (segment_reduce).

Reference semantics:
    lhs = embed[input_ids]                      # only lhs[:, :32, :] is ever used
    m[b,j,x] = (pos[b,j,x] != -1) & (am[b,j] != 0)
    pooled = einsum('bml,bld->bmd', m, lhs[:, :32]) / 32
    x = LayerNorm(pooled) * gamma + beta
    out = (x @ w1) @ w2 + b2                    # [16, 64, 100000]

Device strategy (8 cores, SPMD, no collectives):
  - tensor-parallel over the entity vocab: core c owns w2[:, c*12500:(c+1)*12500]
  - every core redundantly computes hT = (LN(pooled) @ w1).T in [R=128, T=1024]
    layout, then the big projection runs token-tile-stationary:
    out_ps[t, e] = hT[:, ttile].T @ w2[:, echunk].
  - the pooled tensor itself is never materialized on device.  With the
    mask M [x, t] (block-diagonal, scaled by 1/L, built on the host) and
    E = embed rows [x, d]:
        yT   = (E @ w1g).T @ M          (w1g = gamma(.)w1, folded on host)
        mu   = (rowsum(E)/D).T @ M
        s2   = 1_x.T @ ((G @ M) (.) M), G = E @ E.T   (Gram, host-folded)
  - LayerNorm folds the mean-subtraction into a PE outer-product accumulate
    (yT_ps += (-u) (x) mu, u = gamma @ w1) and broadcasts rs = rsqrt(var+eps)
    across partitions with a ones-matmul, so the per-element LN work is just
    3 elementwise passes.  NOTE the has_written discipline: within each PSUM
    bank only the FIRST matmul uses start=True (its whole-bank bit-clear
    makes the second slice's start=False overwrite safe), so the later
    outer-product accumulate sees set bits everywhere.

Main-loop pipeline (sets the 25.6MB/core HBM-write roofline):
  - 100 minigroups of 2 matmul chunks (N=500, one PSUM bank each); PSUM
    cycles through 4 x [128, 2, 512] buffers (8 banks).  DVE evacuates
    buffers 0-1 (banks 0-3), ACT buffers 2-3 (banks 4-7) - the engines never
    touch the same PSUM half, each instruction moves 1000 fp32->bf16 cols
    (~1.0-1.2us), and the 4-deep rotation hides the evac latency that a
    2-buffer scheme serializes on.
  - output DMA destinations are CONTIGUOUS DRAM blocks (the host reassembles
    [T, E]): strided writes measured ~283 GB/s aggregate, contiguous blocks
    ~372 GB/s.  Four col-adjacent minigroups (4000 cols, same tt) share one
    1MB DMA (128 x 8000B descriptors) on the sync HWDGE queue.
  - the per-tt 500-col remainders (cols 12000:12500) are batched 4-tts-at-a-
    time (two minigroups) and written as contiguous 128KB DMAs on the gpsimd
    SWDGE queue.
  - w2 (3.2MB bf16) loads on the sync queue right after the small prefix
    inputs, so reads drain while the prefix computes hT.
"""

import sys

if '/opt/trn_rl_repo' not in sys.path:
    sys.path.insert(0, '/opt/trn_rl_repo')

import numpy as np
import ml_dtypes

import concourse.bass as bass
import concourse.mybir as mybir
import concourse.tile as tile
from concourse import bacc
from concourse.bass_utils import run_bass_kernel_spmd

# model dims (fixed by the problem)
B, S, M, L, D = 16, 512, 64, 32, 1024
V, R, E = 32000, 128, 100000
LN_EPS = 1e-5

N_CORES = 8
T = B * M              # 1024 mention-tokens
ES = E // N_CORES      # 12500 entity columns per core
ECH = 500              # matmul moving chunk (<=512 fp32 PSUM bank)
TT = T // 128          # 8 token tiles
NW2 = 4                # w2 load chunks
W2CH = ES // NW2       # 3125 cols per w2 load
NBLK = TT * 6          # 48 contiguous 512KB output blocks (2 minigroups each)

F32 = mybir.dt.float32
F32R = mybir.dt.float32r    # fp32 data, PE rounds (~tf32)
BF16 = mybir.dt.bfloat16
AF = mybir.AluOpType
ACTF = mybir.ActivationFunctionType


def build_nc(has_b2: bool, has_c: bool):
    nc = bacc.Bacc("TRN2", target_bir_lowering=False, debug=False,
                   enable_asserts=False, num_devices=N_CORES)

    # ---- DRAM I/O (per-core) ----
    d_mblk = nc.dram_tensor("mblk", [128, 4 * 256], BF16, kind="ExternalInput").ap()
    d_gram = nc.dram_tensor("gram", [128, 4 * 128], BF16, kind="ExternalInput").ap()
    d_ew1 = nc.dram_tensor("ew1", [128, 4 * 128], BF16, kind="ExternalInput").ap()
    # packed per-partition smalls: [rsum(4) | ones(2) | c | u | -u] bf16
    d_sm = nc.dram_tensor("smalls", [128, 9], BF16, kind="ExternalInput").ap()
    # packed row vectors: [ones(128) | -u(128)] f32r on partition 0
    d_rv = nc.dram_tensor("rowv", [1, 256], F32R, kind="ExternalInput").ap()
    d_w2 = nc.dram_tensor("w2s", [R, ES], BF16, kind="ExternalInput").ap()
    d_b2 = nc.dram_tensor("b2s", [1, ES], F32R, kind="ExternalInput").ap()
    # contiguous output blocks; the host reassembles [T, ES]:
    #   outb[(6*tt+j)*128 + p, :] = out[tt*128 + p, 2000*j : 2000*(j+1)]
    #   outt[q*128 + p, :]        = out[q*128 + p, 12000:12500]
    d_outb = nc.dram_tensor("outb", [NBLK * 128, 2000], BF16,
                            kind="ExternalOutput").ap()
    d_outt = nc.dram_tensor("outt", [TT * 128, ECH], BF16,
                            kind="ExternalOutput").ap()

    with tile.TileContext(nc) as tc:
        with (
            tc.tile_pool(name="persist", bufs=1) as pp,
            tc.tile_pool(name="pre", bufs=1) as pre,
        ):
            w2_sb = pp.tile([R, ES], BF16)
            hT_sb = pp.tile([R, T], BF16)

            # ---- input DMAs: critical prefix inputs first, then w2, all on
            # the sync HWDGE queue so the small loads aren't round-robin
            # starved by the 3.2MB w2 stream on another queue ----
            mblk_sb = pre.tile([128, T], BF16)
            nc.sync.dma_start(mblk_sb[:], d_mblk[:])
            gram_sb = pre.tile([128, 4, 128], BF16)
            nc.sync.dma_start(gram_sb[:], d_gram[:])
            ew1_sb = pre.tile([128, 4, 128], BF16)
            nc.sync.dma_start(ew1_sb[:], d_ew1[:])
            sm_sb = pre.tile([128, 9], BF16)
            nc.sync.dma_start(sm_sb[:], d_sm[:])
            rv_sb = pre.tile([1, 256], F32R)
            nc.sync.dma_start(rv_sb[:], d_rv[:])
            rsum_sb = sm_sb          # cols 0:4
            ones_sb = sm_sb[:, 4:6]
            cu_sb = sm_sb[:, 6:9]
            onesr_sb = rv_sb[:, 0:128]
            nu_sb = rv_sb[:, 128:256]
            # w2 follows the inputs on the sync queue: FIFO gives the
            # small critical inputs full bandwidth (a parallel SWDGE w2
            # stream starves their tiny descriptors at engine round-robin),
            # and w2 drains before the first output block is ready
            for wc in range(NW2):
                wsl = slice(wc * W2CH, (wc + 1) * W2CH)
                nc.sync.dma_start(w2_sb[:, wsl], d_w2[:, wsl])
            # prime the ACT function tables (Square / Abs_reciprocal_sqrt)
            # so the 1.3us table loads happen during the input DMAs, not on
            # the LN critical path
            scratch_sb = pre.tile([128, 2], F32)
            nc.vector.memset(scratch_sb[:], 1.0)
            nc.scalar.square(scratch_sb[:, 0:1], scratch_sb[:, 1:2])
            nc.scalar.activation(scratch_sb[:, 0:1], scratch_sb[:, 1:2],
                                 ACTF.Abs_reciprocal_sqrt)
            if has_b2:
                b2r_sb = pre.tile([1, ES], F32R)
                nc.scalar.dma_start(b2r_sb[:], d_b2[:])

            # ---- prefix: stats + classifier input, no pooled tensor ----
            with tc.tile_pool(name="preps", bufs=1, space="PSUM") as prps:
                yT_ps = prps.tile([128, T], F32)
                wps_cm = tc.tile_pool(name="wps", bufs=1, space="PSUM")
                wps = wps_cm.__enter__()
                W_ps = wps.tile([128, T], F32)
                s2_ps = wps.tile([1, T], F32)
                mu_ps = wps.tile([1, T], F32)

                # W = G @ M and yT = (E w1g).T @ M and mu = rsum.T @ M
                # yT: only the first 256-slice per bank carries start=True so
                # the has_written bits stay set for the outer accumulate.
                for g in range(4):
                    gsl = slice(g * 256, (g + 1) * 256)
                    nc.tensor.matmul(out=W_ps[:, gsl], lhsT=gram_sb[:, g, :],
                                     rhs=mblk_sb[:, g * 256:(g + 1) * 256], start=True, stop=True,
                                     skip_group_check=True)
                    nc.tensor.matmul(out=yT_ps[:, gsl], lhsT=ew1_sb[:, g, :],
                                     rhs=mblk_sb[:, g * 256:(g + 1) * 256],
                                     start=(g % 2 == 0), stop=False,
                                     skip_group_check=True)
                    nc.tensor.matmul(out=mu_ps[:, gsl], lhsT=rsum_sb[:, g:g + 1],
                                     rhs=mblk_sb[:, g * 256:(g + 1) * 256], start=True, stop=True,
                                     skip_group_check=True)
                # WM = W (.) M  -> s2 = 1.T @ WM
                wm_sb = pre.tile([128, T], BF16)
                mu_sb = pre.tile([1, T], F32R)
                e2_sb = pre.tile([1, T], F32)
                musq_sb = pre.tile([1, T], F32)
                vare_sb = pre.tile([1, T], F32)
                rs_sb = pre.tile([1, T], F32R)
                for nch in range(2):
                    sl = slice(nch * 512, (nch + 1) * 512)
                    nc.vector.tensor_tensor(
                        wm_sb[:, sl], W_ps[:, sl],
                        mblk_sb[:, sl], op=AF.mult)
                    nc.tensor.matmul(out=s2_ps[:, sl], lhsT=ones_sb[:, 0:1],
                                     rhs=wm_sb[:, sl], start=True, stop=True,
                                     skip_group_check=True)
                # per-token scalars on [1, T], halves split across engines
                # (range deps let the h0 chain start after the g=0,1 MMs):
                #   var' = s2/D - mu^2 ; rs = rsqrt(var' + eps)  (eps folded
                #   into the rsqrt bias)
                h0 = slice(0, 512)
                h1 = slice(512, 1024)
                nc.scalar.copy(mu_sb[:, h0], mu_ps[:, h0])
                nc.scalar.square(musq_sb[:, h0], mu_sb[:, h0])
                nc.scalar.copy(mu_sb[:, h1], mu_ps[:, h1])
                # e2eps = s2/D + eps (one TS op), vare = e2eps - mu^2
                nc.vector.tensor_scalar(e2_sb[:, h0], s2_ps[:, h0], 1.0 / D,
                                        LN_EPS, op0=AF.mult, op1=AF.add)
                nc.vector.tensor_tensor(vare_sb[:, h0], e2_sb[:, h0],
                                        musq_sb[:, h0], op=AF.subtract)
                nc.vector.tensor_tensor(musq_sb[:, h1], mu_sb[:, h1],
                                        mu_sb[:, h1], op=AF.mult)
                nc.vector.tensor_scalar(e2_sb[:, h1], s2_ps[:, h1], 1.0 / D,
                                        LN_EPS, op0=AF.mult, op1=AF.add)
                nc.vector.tensor_tensor(vare_sb[:, h1], e2_sb[:, h1],
                                        musq_sb[:, h1], op=AF.subtract)
                # mean-subtraction as PE outer-product accumulate:
                #   yT_ps[r, t] += (-u[r]) * mu[t]
                for nch in range(2):
                    sl = slice(nch * 512, (nch + 1) * 512)
                    nc.tensor.matmul(out=yT_ps[:, sl], lhsT=nu_sb,
                                     rhs=mu_sb[:, sl], start=False, stop=True,
                                     skip_group_check=True)
                # PE warm-up: keep the tensor engine busy through the LN
                # stats phase so HAM reaches K=8/8 before the main stream
                # (results written to the already-consumed W_ps and thrown
                # away; the WAR dep on the wm reads keeps this safe)
                for wu in range(5):
                    nc.tensor.matmul(out=W_ps[:, 0:512], lhsT=gram_sb[:, wu % 4, :],
                                     rhs=mblk_sb[:, 0:512], start=True, stop=True,
                                     skip_group_check=True)
                wps_cm.__exit__(None, None, None)

                # broadcast rs across partitions; hT = rs (.) yT (+ c)
                with tc.tile_pool(name="bcps", bufs=1, space="PSUM") as bps:
                    rsb_ps = bps.tile([128, T], F32)
                    rsb_sb = pre.tile([128, T], F32)
                    if has_c:
                        t1_sb = pre.tile([128, T], BF16)
                    for nch in range(2):
                        sl = slice(nch * 512, (nch + 1) * 512)
                        nc.scalar.activation(rs_sb[:, sl], vare_sb[:, sl],
                                             ACTF.Abs_reciprocal_sqrt)
                        nc.tensor.matmul(out=rsb_ps[:, sl], lhsT=onesr_sb,
                                         rhs=rs_sb[:, sl], start=True, stop=True,
                                         skip_group_check=True)
                        nc.scalar.copy(rsb_sb[:, sl], rsb_ps[:, sl])
                        if has_c:
                            nc.vector.tensor_tensor(t1_sb[:, sl], yT_ps[:, sl],
                                                    rsb_sb[:, sl], op=AF.mult)
                            nc.vector.tensor_scalar(hT_sb[:, sl], t1_sb[:, sl],
                                                    cu_sb[:, 0:1], None, op0=AF.add)
                        else:
                            nc.vector.tensor_tensor(hT_sb[:, sl], yT_ps[:, sl],
                                                    rsb_sb[:, sl], op=AF.mult)

            # ---- main loop: 100 minigroups of 2 x (N=500) chunks ----
            # PSUM pool bufs=4 x [128, 2, 512] (2 banks each).  Minigroup
            # mg -> buffer mg%4; DVE evacuates buffers 0-1, ACT buffers 2-3.
            # Four col-adjacent minigroups (one 4000-col block, same tt)
            # share an o2 tile and one 1MB DMA on sync.  Tail minigroups
            # (cols 12000:12500 across tts) pair up into 128KB SWDGE DMAs.
            mgs = []
            for tt in range(TT):
                for g in range(12):
                    mgs.append(("big", tt, g, [(tt, g * 1000 + k * 500)
                                               for k in range(2)]))
                if tt == 3:
                    mgs.append(("tail", 0, 0, [(0, 12000), (1, 12000)]))
                    mgs.append(("tail", 0, 1, [(2, 12000), (3, 12000)]))
                if tt == 7:
                    mgs.append(("tail", 1, 0, [(4, 12000), (5, 12000)]))
                    mgs.append(("tail", 1, 1, [(6, 12000), (7, 12000)]))

            with tc.tile_pool(name="mainps", bufs=4, space="PSUM") as mps, \
                 tc.tile_pool(name="outp", bufs=8) as op, \
                 tc.tile_pool(name="outtp", bufs=2) as otp:
                # phase-shift the PSUM rotation by two buffers so the first
                # minigroups land on the banks the prefix frees earliest
                # (banks 4-7: s2/mu) instead of banks 0-1, whose last reader
                # (the hT multiply against yT_ps) is the final prefix op
                ps_skip0 = mps.tile([128, 2, 512], F32, tag="ps", name="ps_skip0")
                ps_skip1 = mps.tile([128, 2, 512], F32, tag="ps", name="ps_skip1")
                o2p = None
                o2t = None
                for mg, (kind, tt, g, chunks) in enumerate(mgs):
                    ps = mps.tile([128, 2, 512], F32, tag="ps")
                    for k, (ctt, cs) in enumerate(chunks):
                        nc.tensor.matmul(out=ps[:, k, 0:ECH],
                                         lhsT=hT_sb[:, ctt * 128:(ctt + 1) * 128],
                                         rhs=w2_sb[:, cs:cs + ECH],
                                         start=True, stop=not has_b2)
                        if has_b2:
                            nc.tensor.matmul(out=ps[:, k, 0:ECH],
                                             lhsT=onesr_sb,
                                             rhs=b2r_sb[:, cs:cs + ECH],
                                             start=False, stop=True)
                    # evac engine follows the PSUM buffer (mg%4):
                    # DVE owns buffers 0-1 (banks 0-3), ACT 2-3 (banks 4-7)
                    evac = (nc.vector.tensor_copy if mg % 4 in (0, 1)
                            else nc.scalar.copy)
                    if kind == "big":
                        if g % 2 == 0:
                            o2p = op.tile([128, 2, 2, ECH], BF16, tag="o2",
                                          name=f"o2_{mg}")
                        evac(o2p[:, g % 2, :, :], ps[:, :, 0:ECH])
                        if g % 2 == 1:
                            blk = 6 * tt + g // 2
                            nc.sync.dma_start(
                                d_outb[blk * 128:(blk + 1) * 128, :],
                                o2p[:])
                            o2p = None
                    else:
                        if g == 0:
                            o2t = otp.tile([128, 2, 2, ECH], BF16, tag="o2t",
                                           name=f"o2t_{mg}")
                        evac(o2t[:, g, :, :], ps[:, :, 0:ECH])
                        if g == 1:
                            for q in range(4):
                                qq = tt * 4 + q
                                nc.gpsimd.dma_start(
                                    d_outt[qq * 128:(qq + 1) * 128, :],
                                    o2t[:, q // 2, q % 2, :])
                            o2t = None

    nc.finalize()
    return nc


_NC_CACHE = {}


def _get_nc(has_b2: bool, has_c: bool):
    key = (has_b2, has_c)
    if key not in _NC_CACHE:
        _NC_CACHE[key] = build_nc(has_b2, has_c)
    return _NC_CACHE[key]


def prep_core_inputs(inputs):
    """Host-side sharding/layout prep. Returns (shared_map, per_core_w2, per_core_b2, has_b2, has_c)."""
    ids = np.asarray(inputs["input_ids"]).astype(np.int32)[:, :L]      # [16, 32]
    pos = np.asarray(inputs["entity_position_ids"]).astype(np.int32)   # [16, 64, 32]
    am = np.asarray(inputs["entity_attention_mask"]).astype(np.int32)  # [16, 64]
    embed = np.ascontiguousarray(np.asarray(inputs["embed"], dtype=np.float32))
    gamma = np.asarray(inputs["ln_gamma"], dtype=np.float32)
    beta = np.asarray(inputs["ln_beta"], dtype=np.float32)
    w1 = np.ascontiguousarray(np.asarray(inputs["w1"], dtype=np.float32))
    w2 = np.asarray(inputs["w2"], dtype=np.float32)
    b2 = np.asarray(inputs["b2"], dtype=np.float32)

    # E_g[p=32b+x, g, :] = embed[ids[4g+b, x]]  (batch 4g+b on k-block b)
    emb_idx = ids.reshape(4, 4, L).transpose(1, 2, 0).reshape(128, 4)
    emb_g = embed[emb_idx]                                  # [128, 4, D]
    rsum = np.ascontiguousarray(
        emb_g.sum(axis=2, dtype=np.float64).astype(np.float32) / D)   # [128, 4]

    # block-diagonal mask, scaled by 1/L:
    #   mblk[32k+x, g, 64k+j] = m[4g+k, j, x] / L
    mask = (((pos != -1) & (am[:, :, None] != 0)).astype(np.float32) / L)  # [b,j,x]
    mblk = np.zeros((128, 4, 256), np.float32)
    for g in range(4):
        for k in range(4):
            mblk[32 * k:32 * (k + 1), g, 64 * k:64 * (k + 1)] = \
                mask[4 * g + k].T
    mblk = np.ascontiguousarray(mblk.reshape(128, 4 * 256))

    # host-folded small matrices (mask-independent):
    #   gram[:, g, :] = E_g @ E_g.T ; ew1[:, g, :] = E_g @ (gamma (.) w1)
    w1g = gamma[:, None] * w1                               # [D, R]
    eg64 = emb_g.astype(np.float64)
    gram = np.einsum('pgd,qgd->pgq', eg64, eg64).astype(np.float32)
    ew1 = np.einsum('pgd,dr->pgr', eg64, w1g.astype(np.float64)).astype(np.float32)
    u = gamma @ w1                                          # [R]
    cvec = beta @ w1                                        # [R]
    cu = np.ascontiguousarray(
        np.stack([cvec, u, -u], axis=-1))                   # [R, 3]

    bf = ml_dtypes.bfloat16
    sm = np.concatenate([rsum, np.ones((128, 2), np.float32), cu], axis=1)
    rv = np.concatenate([np.ones(128, np.float32), -u]).reshape(1, 256)
    shared = {
        "mblk": mblk.astype(bf),
        "gram": np.ascontiguousarray(gram.reshape(128, 4 * 128)).astype(bf),
        "ew1": np.ascontiguousarray(ew1.reshape(128, 4 * 128)).astype(bf),
        "smalls": np.ascontiguousarray(sm).astype(bf),
        "rowv": np.ascontiguousarray(rv),
    }
    w2s = [np.ascontiguousarray(w2[:, c * ES:(c + 1) * ES]).astype(ml_dtypes.bfloat16)
           for c in range(N_CORES)]
    b2s = [np.ascontiguousarray(b2[c * ES:(c + 1) * ES].reshape(1, ES))
           for c in range(N_CORES)]
    has_b2 = bool(np.any(b2 != 0.0))
    has_c = bool(np.any(cvec != 0.0))
    return shared, w2s, b2s, has_b2, has_c


def kernel(**inputs) -> np.ndarray:
    shared, w2s, b2s, has_b2, has_c = prep_core_inputs(inputs)
    nc = _get_nc(has_b2, has_c)
    in_maps = [dict(shared, w2s=w2s[c], b2s=b2s[c]) for c in range(N_CORES)]
    res = run_bass_kernel_spmd(nc, in_maps, list(range(N_CORES)))
    full = np.empty((T, E), np.float32)
    for c in range(N_CORES):
        co = c * ES
        outb = np.asarray(res.results[c]["outb"]).astype(np.float32)
        outt = np.asarray(res.results[c]["outt"]).astype(np.float32)
        # outb[(3*tt+j)*128 + p, :] -> full[tt*128 + p, co + 4000*j : +4000]
        ob = outb.reshape(TT, 6, 128, 2000).transpose(0, 2, 1, 3).reshape(T, 12000)
        full[:, co:co + 12000] = ob
        full[:, co + 12000:co + 12500] = outt
    return np.ascontiguousarray(full.reshape(B, M, E))


# revision 15
# speedup vs baseline: 1.0235x; 1.0235x over previous
"""Trainium2 Bass kernel for nn_EnokeeEncoder (segment_reduce).

Reference semantics:
    lhs = embed[input_ids]                      # only lhs[:, :32, :] is ever used
    m[b,j,x] = (pos[b,j,x] != -1) & (am[b,j] != 0)
    pooled = einsum('bml,bld->bmd', m, lhs[:, :32]) / 32
    x = LayerNorm(pooled) * gamma + beta
    out = (x @ w1) @ w2 + b2                    # [16, 64, 100000]

Device strategy (8 cores, SPMD, no collectives):
  - tensor-parallel over the entity vocab: core c owns w2[:, c*12500:(c+1)*12500]
  - every core redundantly computes hT = (LN(pooled) @ w1).T in [R=128, T=1024]
    layout, then the big projection runs token-tile-stationary:
    out_ps[t, e] = hT[:, ttile].T @ w2[:, echunk].
  - the pooled tensor itself is never materialized on device.  With the
    mask M [x, t] (block-diagonal, scaled by 1/L, built on the host) and
    E = embed rows [x, d]:
        yT   = (E @ w1g).T @ M          (w1g = gamma(.)w1, folded on host)
        mu   = (rowsum(E)/D).T @ M
        s2   = 1_x.T @ ((G @ M) (.) M), G = E @ E.T   (Gram, host-folded)
  - LayerNorm folds the mean-subtraction into a PE outer-product accumulate
    (yT_ps += (-u) (x) mu, u = gamma @ w1) and broadcasts rs = rsqrt(var+eps)
    across partitions with a ones-matmul, so the per-element LN work is just
    3 elementwise passes.  NOTE the has_written discipline: within each PSUM
    bank only the FIRST matmul uses start=True (its whole-bank bit-clear
    makes the second slice's start=False overwrite safe), so the later
    outer-product accumulate sees set bits everywhere.

Main-loop pipeline (sets the 25.6MB/core HBM-write roofline):
  - 100 minigroups of 2 matmul chunks (N=500, one PSUM bank each); PSUM
    cycles through 4 x [128, 2, 512] buffers (8 banks).  DVE evacuates
    buffers 0-1 (banks 0-3), ACT buffers 2-3 (banks 4-7) - the engines never
    touch the same PSUM half, each instruction moves 1000 fp32->bf16 cols
    (~1.0-1.2us), and the 4-deep rotation hides the evac latency that a
    2-buffer scheme serializes on.
  - output DMA destinations are CONTIGUOUS DRAM blocks (the host reassembles
    [T, E]): strided writes measured ~283 GB/s aggregate, contiguous blocks
    ~372 GB/s.  Four col-adjacent minigroups (4000 cols, same tt) share one
    1MB DMA (128 x 8000B descriptors) on the sync HWDGE queue.
  - the per-tt 500-col remainders (cols 12000:12500) are batched 4-tts-at-a-
    time (two minigroups) and written as contiguous 128KB DMAs on the gpsimd
    SWDGE queue.
  - w2 (3.2MB bf16) loads on the sync queue right after the small prefix
    inputs, so reads drain while the prefix computes hT.
"""

import sys

if '/opt/trn_rl_repo' not in sys.path:
    sys.path.insert(0, '/opt/trn_rl_repo')

import numpy as np
import ml_dtypes

import concourse.bass as bass
import concourse.mybir as mybir
import concourse.tile as tile
from concourse import bacc
from concourse.bass_utils import run_bass_kernel_spmd

# model dims (fixed by the problem)
B, S, M, L, D = 16, 512, 64, 32, 1024
V, R, E = 32000, 128, 100000
LN_EPS = 1e-5

N_CORES = 8
T = B * M              # 1024 mention-tokens
ES = E // N_CORES      # 12500 entity columns per core
ECH = 500              # matmul moving chunk (<=512 fp32 PSUM bank)
TT = T // 128          # 8 token tiles
NW2 = 4                # w2 load chunks
W2CH = ES // NW2       # 3125 cols per w2 load
NBLK = TT * 6          # 48 contiguous 512KB output blocks (2 minigroups each)

F32 = mybir.dt.float32
F32R = mybir.dt.float32r    # fp32 data, PE rounds (~tf32)
BF16 = mybir.dt.bfloat16
AF = mybir.AluOpType
ACTF = mybir.ActivationFunctionType


def build_nc(has_b2: bool, has_c: bool):
    nc = bacc.Bacc("TRN2", target_bir_lowering=False, debug=False,
                   enable_asserts=False, num_devices=N_CORES)

    # ---- DRAM I/O (per-core) ----
    d_mblk = nc.dram_tensor("mblk", [128, 4 * 256], BF16, kind="ExternalInput").ap()
    d_gram = nc.dram_tensor("gram", [128, 4 * 128], BF16, kind="ExternalInput").ap()
    d_ew1 = nc.dram_tensor("ew1", [128, 4 * 128], BF16, kind="ExternalInput").ap()
    # packed per-partition smalls: [rsum(4) | ones(2) | c | u | -u] bf16
    d_sm = nc.dram_tensor("smalls", [128, 9], BF16, kind="ExternalInput").ap()
    # packed row vectors: [ones(128) | -u(128)] f32r on partition 0
    d_rv = nc.dram_tensor("rowv", [1, 256], F32R, kind="ExternalInput").ap()
    d_w2 = nc.dram_tensor("w2s", [R, ES], BF16, kind="ExternalInput").ap()
    d_b2 = nc.dram_tensor("b2s", [1, ES], F32R, kind="ExternalInput").ap()
    # contiguous output blocks; the host reassembles [T, ES]:
    #   outb[(6*tt+j)*128 + p, :] = out[tt*128 + p, 2000*j : 2000*(j+1)]
    #   outt[q*128 + p, :]        = out[q*128 + p, 12000:12500]
    d_outb = nc.dram_tensor("outb", [NBLK * 128, 2000], BF16,
                            kind="ExternalOutput").ap()
    d_outt = nc.dram_tensor("outt", [TT * 128, ECH], BF16,
                            kind="ExternalOutput").ap()

    with tile.TileContext(nc) as tc:
        with (
            tc.tile_pool(name="persist", bufs=1) as pp,
            tc.tile_pool(name="pre", bufs=1) as pre,
        ):
            w2_sb = pp.tile([R, ES], BF16)
            hT_sb = pp.tile([R, T], BF16)

            # ---- input DMAs: critical prefix inputs first, then w2, all on
            # the sync HWDGE queue so the small loads aren't round-robin
            # starved by the 3.2MB w2 stream on another queue ----
            mblk_sb = pre.tile([128, T], BF16)
            nc.sync.dma_start(mblk_sb[:], d_mblk[:])
            gram_sb = pre.tile([128, 4, 128], BF16)
            nc.sync.dma_start(gram_sb[:], d_gram[:])
            ew1_sb = pre.tile([128, 4, 128], BF16)
            nc.sync.dma_start(ew1_sb[:], d_ew1[:])
            sm_sb = pre.tile([128, 9], BF16)
            nc.sync.dma_start(sm_sb[:], d_sm[:])
            rv_sb = pre.tile([1, 256], F32R)
            nc.sync.dma_start(rv_sb[:], d_rv[:])
            rsum_sb = sm_sb          # cols 0:4
            ones_sb = sm_sb[:, 4:6]
            cu_sb = sm_sb[:, 6:9]
            onesr_sb = rv_sb[:, 0:128]
            nu_sb = rv_sb[:, 128:256]
            # w2 follows the inputs on the sync queue: FIFO gives the
            # small critical inputs full bandwidth (a parallel SWDGE w2
            # stream starves their tiny descriptors at engine round-robin),
            # and w2 drains before the first output block is ready
            for wc in range(NW2):
                wsl = slice(wc * W2CH, (wc + 1) * W2CH)
                nc.sync.dma_start(w2_sb[:, wsl], d_w2[:, wsl])
            # prime the ACT function tables (Square / Abs_reciprocal_sqrt)
            # so the 1.3us table loads happen during the input DMAs, not on
            # the LN critical path
            scratch_sb = pre.tile([128, 2], F32)
            nc.vector.memset(scratch_sb[:], 1.0)
            nc.scalar.square(scratch_sb[:, 0:1], scratch_sb[:, 1:2])
            nc.scalar.activation(scratch_sb[:, 0:1], scratch_sb[:, 1:2],
                                 ACTF.Abs_reciprocal_sqrt)
            if has_b2:
                b2r_sb = pre.tile([1, ES], F32R)
                nc.scalar.dma_start(b2r_sb[:], d_b2[:])

            # ---- prefix: stats + classifier input, no pooled tensor ----
            with tc.tile_pool(name="preps", bufs=1, space="PSUM") as prps:
                yT_ps = prps.tile([128, T], F32)
                wps_cm = tc.tile_pool(name="wps", bufs=1, space="PSUM")
                wps = wps_cm.__enter__()
                W_ps = wps.tile([128, T], F32)
                s2_ps = wps.tile([1, T], F32)
                mu_ps = wps.tile([1, T], F32)

                # W = G @ M and yT = (E w1g).T @ M and mu = rsum.T @ M
                # yT: only the first 256-slice per bank carries start=True so
                # the has_written bits stay set for the outer accumulate.
                for g in range(4):
                    gsl = slice(g * 256, (g + 1) * 256)
                    nc.tensor.matmul(out=W_ps[:, gsl], lhsT=gram_sb[:, g, :],
                                     rhs=mblk_sb[:, g * 256:(g + 1) * 256], start=True, stop=True,
                                     skip_group_check=True)
                    nc.tensor.matmul(out=yT_ps[:, gsl], lhsT=ew1_sb[:, g, :],
                                     rhs=mblk_sb[:, g * 256:(g + 1) * 256],
                                     start=(g % 2 == 0), stop=False,
                                     skip_group_check=True)
                    nc.tensor.matmul(out=mu_ps[:, gsl], lhsT=rsum_sb[:, g:g + 1],
                                     rhs=mblk_sb[:, g * 256:(g + 1) * 256], start=True, stop=True,
                                     skip_group_check=True)
                # WM = W (.) M  -> s2 = 1.T @ WM
                wm_sb = pre.tile([128, T], BF16)
                mu_sb = pre.tile([1, T], F32R)
                e2_sb = pre.tile([1, T], F32)
                musq_sb = pre.tile([1, T], F32)
                vare_sb = pre.tile([1, T], F32)
                rs_sb = pre.tile([1, T], F32R)
                for nch in range(2):
                    sl = slice(nch * 512, (nch + 1) * 512)
                    nc.vector.tensor_tensor(
                        wm_sb[:, sl], W_ps[:, sl],
                        mblk_sb[:, sl], op=AF.mult)
                    nc.tensor.matmul(out=s2_ps[:, sl], lhsT=ones_sb[:, 0:1],
                                     rhs=wm_sb[:, sl], start=True, stop=True,
                                     skip_group_check=True)
                # per-token scalars on [1, T], halves split across engines
                # (range deps let the h0 chain start after the g=0,1 MMs):
                #   var' = s2/D - mu^2 ; rs = rsqrt(var' + eps)  (eps folded
                #   into the rsqrt bias)
                h0 = slice(0, 512)
                h1 = slice(512, 1024)
                nc.scalar.copy(mu_sb[:, h0], mu_ps[:, h0])
                nc.scalar.square(musq_sb[:, h0], mu_sb[:, h0])
                # h1 mu copy rides DVE: the ACT queue (mu/square/rsqrt/rsb
                # copies) is the serial tail of the prefix, and banks 0-1
                # stay blocked for the main loop until it drains
                nc.vector.tensor_copy(mu_sb[:, h1], mu_ps[:, h1])
                # e2eps = s2/D + eps (one TS op), vare = e2eps - mu^2
                nc.vector.tensor_scalar(e2_sb[:, h0], s2_ps[:, h0], 1.0 / D,
                                        LN_EPS, op0=AF.mult, op1=AF.add)
                nc.vector.tensor_tensor(vare_sb[:, h0], e2_sb[:, h0],
                                        musq_sb[:, h0], op=AF.subtract)
                nc.vector.tensor_tensor(musq_sb[:, h1], mu_sb[:, h1],
                                        mu_sb[:, h1], op=AF.mult)
                nc.vector.tensor_scalar(e2_sb[:, h1], s2_ps[:, h1], 1.0 / D,
                                        LN_EPS, op0=AF.mult, op1=AF.add)
                nc.vector.tensor_tensor(vare_sb[:, h1], e2_sb[:, h1],
                                        musq_sb[:, h1], op=AF.subtract)
                # mean-subtraction as PE outer-product accumulate:
                #   yT_ps[r, t] += (-u[r]) * mu[t]
                for nch in range(2):
                    sl = slice(nch * 512, (nch + 1) * 512)
                    nc.tensor.matmul(out=yT_ps[:, sl], lhsT=nu_sb,
                                     rhs=mu_sb[:, sl], start=False, stop=True,
                                     skip_group_check=True)
                # PE warm-up: keep the tensor engine busy through the LN
                # stats phase so HAM reaches K=8/8 before the main stream
                # (results written to the already-consumed W_ps and thrown
                # away; the WAR dep on the wm reads keeps this safe)
                for wu in range(5):
                    nc.tensor.matmul(out=W_ps[:, 0:512], lhsT=gram_sb[:, wu % 4, :],
                                     rhs=mblk_sb[:, 0:512], start=True, stop=True,
                                     skip_group_check=True)
                wps_cm.__exit__(None, None, None)

                # broadcast rs across partitions; hT = rs (.) yT (+ c)
                with tc.tile_pool(name="bcps", bufs=1, space="PSUM") as bps:
                    rsb_ps = bps.tile([128, T], F32)
                    rsb_sb = pre.tile([128, T], F32)
                    if has_c:
                        t1_sb = pre.tile([128, T], BF16)
                    for nch in range(2):
                        sl = slice(nch * 512, (nch + 1) * 512)
                        nc.scalar.activation(rs_sb[:, sl], vare_sb[:, sl],
                                             ACTF.Abs_reciprocal_sqrt)
                        nc.tensor.matmul(out=rsb_ps[:, sl], lhsT=onesr_sb,
                                         rhs=rs_sb[:, sl], start=True, stop=True,
                                         skip_group_check=True)
                        nc.scalar.copy(rsb_sb[:, sl], rsb_ps[:, sl])
                        if has_c:
                            nc.vector.tensor_tensor(t1_sb[:, sl], yT_ps[:, sl],
                                                    rsb_sb[:, sl], op=AF.mult)
                            nc.vector.tensor_scalar(hT_sb[:, sl], t1_sb[:, sl],
                                                    cu_sb[:, 0:1], None, op0=AF.add)
                        else:
                            nc.vector.tensor_tensor(hT_sb[:, sl], yT_ps[:, sl],
                                                    rsb_sb[:, sl], op=AF.mult)

            # ---- main loop: 100 minigroups of 2 x (N=500) chunks ----
            # PSUM pool bufs=4 x [128, 2, 512] (2 banks each).  Minigroup
            # mg -> buffer mg%4; DVE evacuates buffers 0-1, ACT buffers 2-3.
            # Four col-adjacent minigroups (one 4000-col block, same tt)
            # share an o2 tile and one 1MB DMA on sync.  Tail minigroups
            # (cols 12000:12500 across tts) pair up into 128KB SWDGE DMAs.
            mgs = []
            for tt in range(TT):
                for g in range(12):
                    mgs.append(("big", tt, g, [(tt, g * 1000 + k * 500)
                                               for k in range(2)]))
                if tt == 3:
                    mgs.append(("tail", 0, 0, [(0, 12000), (1, 12000)]))
                    mgs.append(("tail", 0, 1, [(2, 12000), (3, 12000)]))
                if tt == 7:
                    mgs.append(("tail", 1, 0, [(4, 12000), (5, 12000)]))
                    mgs.append(("tail", 1, 1, [(6, 12000), (7, 12000)]))

            with tc.tile_pool(name="mainps", bufs=4, space="PSUM") as mps, \
                 tc.tile_pool(name="outp", bufs=8) as op, \
                 tc.tile_pool(name="outtp", bufs=2) as otp:
                o2p = None
                o2t = None
                for mg, (kind, tt, g, chunks) in enumerate(mgs):
                    ps = mps.tile([128, 2, 512], F32, tag="ps")
                    for k, (ctt, cs) in enumerate(chunks):
                        nc.tensor.matmul(out=ps[:, k, 0:ECH],
                                         lhsT=hT_sb[:, ctt * 128:(ctt + 1) * 128],
                                         rhs=w2_sb[:, cs:cs + ECH],
                                         start=True, stop=not has_b2)
                        if has_b2:
                            nc.tensor.matmul(out=ps[:, k, 0:ECH],
                                             lhsT=onesr_sb,
                                             rhs=b2r_sb[:, cs:cs + ECH],
                                             start=False, stop=True)
                    # evac engine follows the PSUM buffer (mg%4):
                    # DVE owns buffers 0-1 (banks 0-3), ACT 2-3 (banks 4-7)
                    evac = (nc.vector.tensor_copy if mg % 4 in (0, 1)
                            else nc.scalar.copy)
                    if kind == "big":
                        if g % 2 == 0:
                            o2p = op.tile([128, 2, 2, ECH], BF16, tag="o2",
                                          name=f"o2_{mg}")
                        evac(o2p[:, g % 2, :, :], ps[:, :, 0:ECH])
                        if g % 2 == 1:
                            blk = 6 * tt + g // 2
                            nc.sync.dma_start(
                                d_outb[blk * 128:(blk + 1) * 128, :],
                                o2p[:])
                            o2p = None
                    else:
                        if g == 0:
                            o2t = otp.tile([128, 2, 2, ECH], BF16, tag="o2t",
                                           name=f"o2t_{mg}")
                        evac(o2t[:, g, :, :], ps[:, :, 0:ECH])
                        if g == 1:
                            for q in range(4):
                                qq = tt * 4 + q
                                nc.gpsimd.dma_start(
                                    d_outt[qq * 128:(qq + 1) * 128, :],
                                    o2t[:, q // 2, q % 2, :])
                            o2t = None

    nc.finalize()
    return nc


_NC_CACHE = {}


def _get_nc(has_b2: bool, has_c: bool):
    key = (has_b2, has_c)
    if key not in _NC_CACHE:
        _NC_CACHE[key] = build_nc(has_b2, has_c)
    return _NC_CACHE[key]


def prep_core_inputs(inputs):
    """Host-side sharding/layout prep. Returns (shared_map, per_core_w2, per_core_b2, has_b2, has_c)."""
    ids = np.asarray(inputs["input_ids"]).astype(np.int32)[:, :L]      # [16, 32]
    pos = np.asarray(inputs["entity_position_ids"]).astype(np.int32)   # [16, 64, 32]
    am = np.asarray(inputs["entity_attention_mask"]).astype(np.int32)  # [16, 64]
    embed = np.ascontiguousarray(np.asarray(inputs["embed"], dtype=np.float32))
    gamma = np.asarray(inputs["ln_gamma"], dtype=np.float32)
    beta = np.asarray(inputs["ln_beta"], dtype=np.float32)
    w1 = np.ascontiguousarray(np.asarray(inputs["w1"], dtype=np.float32))
    w2 = np.asarray(inputs["w2"], dtype=np.float32)
    b2 = np.asarray(inputs["b2"], dtype=np.float32)

    # E_g[p=32b+x, g, :] = embed[ids[4g+b, x]]  (batch 4g+b on k-block b)
    emb_idx = ids.reshape(4, 4, L).transpose(1, 2, 0).reshape(128, 4)
    emb_g = embed[emb_idx]                                  # [128, 4, D]
    rsum = np.ascontiguousarray(
        emb_g.sum(axis=2, dtype=np.float64).astype(np.float32) / D)   # [128, 4]

    # block-diagonal mask, scaled by 1/L:
    #   mblk[32k+x, g, 64k+j] = m[4g+k, j, x] / L
    mask = (((pos != -1) & (am[:, :, None] != 0)).astype(np.float32) / L)  # [b,j,x]
    mblk = np.zeros((128, 4, 256), np.float32)
    for g in range(4):
        for k in range(4):
            mblk[32 * k:32 * (k + 1), g, 64 * k:64 * (k + 1)] = \
                mask[4 * g + k].T
    mblk = np.ascontiguousarray(mblk.reshape(128, 4 * 256))

    # host-folded small matrices (mask-independent):
    #   gram[:, g, :] = E_g @ E_g.T ; ew1[:, g, :] = E_g @ (gamma (.) w1)
    w1g = gamma[:, None] * w1                               # [D, R]
    eg64 = emb_g.astype(np.float64)
    gram = np.einsum('pgd,qgd->pgq', eg64, eg64).astype(np.float32)
    ew1 = np.einsum('pgd,dr->pgr', eg64, w1g.astype(np.float64)).astype(np.float32)
    u = gamma @ w1                                          # [R]
    cvec = beta @ w1                                        # [R]
    cu = np.ascontiguousarray(
        np.stack([cvec, u, -u], axis=-1))                   # [R, 3]

    bf = ml_dtypes.bfloat16
    sm = np.concatenate([rsum, np.ones((128, 2), np.float32), cu], axis=1)
    rv = np.concatenate([np.ones(128, np.float32), -u]).reshape(1, 256)
    shared = {
        "mblk": mblk.astype(bf),
        "gram": np.ascontiguousarray(gram.reshape(128, 4 * 128)).astype(bf),
        "ew1": np.ascontiguousarray(ew1.reshape(128, 4 * 128)).astype(bf),
        "smalls": np.ascontiguousarray(sm).astype(bf),
        "rowv": np.ascontiguousarray(rv),
    }
    w2s = [np.ascontiguousarray(w2[:, c * ES:(c + 1) * ES]).astype(ml_dtypes.bfloat16)
           for c in range(N_CORES)]
    b2s = [np.ascontiguousarray(b2[c * ES:(c + 1) * ES].reshape(1, ES))
           for c in range(N_CORES)]
    has_b2 = bool(np.any(b2 != 0.0))
    has_c = bool(np.any(cvec != 0.0))
    return shared, w2s, b2s, has_b2, has_c


def kernel(**inputs) -> np.ndarray:
    shared, w2s, b2s, has_b2, has_c = prep_core_inputs(inputs)
    nc = _get_nc(has_b2, has_c)
    in_maps = [dict(shared, w2s=w2s[c], b2s=b2s[c]) for c in range(N_CORES)]
    res = run_bass_kernel_spmd(nc, in_maps, list(range(N_CORES)))
    full = np.empty((T, E), np.float32)
    for c in range(N_CORES):
        co = c * ES
        outb = np.asarray(res.results[c]["outb"]).astype(np.float32)
        outt = np.asarray(res.results[c]["outt"]).astype(np.float32)
        # outb[(3*tt+j)*128 + p, :] -> full[tt*128 + p, co + 4000*j : +4000]
        ob = outb.reshape(TT, 6, 128, 2000).transpose(0, 2, 1, 3).reshape(T, 12000)
        full[:, co:co + 12000] = ob
        full[:, co + 12000:co + 12500] = outt
    return np.ascontiguousarray(full.reshape(B, M, E))


# revision 17
# speedup vs baseline: 1.1086x; 1.0832x over previous
"""Trainium2 Bass kernel for nn_EnokeeEncoder (segment_reduce).

Reference semantics:
    lhs = embed[input_ids]                      # only lhs[:, :32, :] is ever used
    m[b,j,x] = (pos[b,j,x] != -1) & (am[b,j] != 0)
    pooled = einsum('bml,bld->bmd', m, lhs[:, :32]) / 32
    x = LayerNorm(pooled) * gamma + beta
    out = (x @ w1) @ w2 + b2                    # [16, 64, 100000]

Device strategy (8 cores, SPMD, no collectives):
  - tensor-parallel over the entity vocab: core c owns w2[:, c*12500:(c+1)*12500]
  - every core redundantly computes hT = (LN(pooled) @ w1).T in [R=128, T=1024]
    layout, then the big projection runs token-tile-stationary:
    out_ps[t, e] = hT[:, ttile].T @ w2[:, echunk].
  - the pooled tensor itself is never materialized on device.  With the
    mask M [x, t] (block-diagonal, scaled by 1/L, built on the host) and
    E = embed rows [x, d]:
        yT   = (E @ w1g).T @ M          (w1g = gamma(.)w1, folded on host)
        mu   = (rowsum(E)/D).T @ M
        s2   = 1_x.T @ ((G @ M) (.) M), G = E @ E.T   (Gram, host-folded)
  - LayerNorm folds the mean-subtraction into a PE outer-product accumulate
    (yT_ps += (-u) (x) mu, u = gamma @ w1) and broadcasts rs = rsqrt(var+eps)
    across partitions with a ones-matmul, so the per-element LN work is just
    3 elementwise passes.  NOTE the has_written discipline: within each PSUM
    bank only the FIRST matmul uses start=True (its whole-bank bit-clear
    makes the second slice's start=False overwrite safe), so the later
    outer-product accumulate sees set bits everywhere.

Main-loop pipeline (sets the 25.6MB/core HBM-write roofline):
  - 100 minigroups of 2 matmul chunks (N=500, one PSUM bank each); PSUM
    cycles through 4 x [128, 2, 512] buffers (8 banks).  DVE evacuates
    buffers 0-1 (banks 0-3), ACT buffers 2-3 (banks 4-7) - the engines never
    touch the same PSUM half, each instruction moves 1000 fp32->bf16 cols
    (~1.0-1.2us), and the 4-deep rotation hides the evac latency that a
    2-buffer scheme serializes on.
  - output DMA destinations are CONTIGUOUS DRAM blocks (the host reassembles
    [T, E]): strided writes measured ~283 GB/s aggregate, contiguous blocks
    ~372 GB/s.  Four col-adjacent minigroups (4000 cols, same tt) share one
    1MB DMA (128 x 8000B descriptors) on the sync HWDGE queue.
  - the per-tt 500-col remainders (cols 12000:12500) are batched 4-tts-at-a-
    time (two minigroups) and written as contiguous 128KB DMAs on the gpsimd
    SWDGE queue.
  - w2 (3.2MB bf16) loads on the sync queue right after the small prefix
    inputs, so reads drain while the prefix computes hT.
"""

import sys

if '/opt/trn_rl_repo' not in sys.path:
    sys.path.insert(0, '/opt/trn_rl_repo')

import numpy as np
import ml_dtypes

import concourse.bass as bass
import concourse.mybir as mybir
import concourse.tile as tile
from concourse import bacc
from concourse.bass_utils import run_bass_kernel_spmd

# model dims (fixed by the problem)
B, S, M, L, D = 16, 512, 64, 32, 1024
V, R, E = 32000, 128, 100000
LN_EPS = 1e-5

N_CORES = 8
T = B * M              # 1024 mention-tokens
ES = E // N_CORES      # 12500 entity columns per core
ECH = 500              # matmul moving chunk (<=512 fp32 PSUM bank)
TT = T // 128          # 8 token tiles
NW2 = 4                # w2 load chunks
W2CH = ES // NW2       # 3125 cols per w2 load
NBLK = TT * 6          # 48 contiguous 512KB output blocks (2 minigroups each)

F32 = mybir.dt.float32
F32R = mybir.dt.float32r    # fp32 data, PE rounds (~tf32)
BF16 = mybir.dt.bfloat16
AF = mybir.AluOpType
ACTF = mybir.ActivationFunctionType


def build_nc(has_b2: bool, has_c: bool):
    nc = bacc.Bacc("TRN2", target_bir_lowering=False, debug=False,
                   enable_asserts=False, num_devices=N_CORES)

    # ---- DRAM I/O (per-core) ----
    d_mblk = nc.dram_tensor("mblk", [128, 4 * 256], BF16, kind="ExternalInput").ap()
    d_gram = nc.dram_tensor("gram", [128, 4 * 128], BF16, kind="ExternalInput").ap()
    d_ew1 = nc.dram_tensor("ew1", [128, 4 * 128], BF16, kind="ExternalInput").ap()
    # packed per-partition smalls: [rsum(4) | ones(2) | c | u | -u] bf16
    d_sm = nc.dram_tensor("smalls", [128, 9], BF16, kind="ExternalInput").ap()
    # packed row vectors: [ones(128) | -u(128)] f32r on partition 0
    d_rv = nc.dram_tensor("rowv", [1, 256], F32R, kind="ExternalInput").ap()
    d_w2 = nc.dram_tensor("w2s", [R, ES], BF16, kind="ExternalInput").ap()
    d_b2 = nc.dram_tensor("b2s", [1, ES], F32R, kind="ExternalInput").ap()
    # contiguous output blocks; the host reassembles [T, ES]:
    #   outb[(6*tt+j)*128 + p, :] = out[tt*128 + p, 2000*j : 2000*(j+1)]
    #   outt[q*128 + p, :]        = out[q*128 + p, 12000:12500]
    d_outb = nc.dram_tensor("outb", [NBLK * 128, 2000], BF16,
                            kind="ExternalOutput").ap()
    d_outt = nc.dram_tensor("outt", [TT * 128, ECH], BF16,
                            kind="ExternalOutput").ap()

    with tile.TileContext(nc) as tc:
        with (
            tc.tile_pool(name="persist", bufs=1) as pp,
            tc.tile_pool(name="pre", bufs=1) as pre,
        ):
            w2_sb = pp.tile([R, ES], BF16)
            hT_sb = pp.tile([R, T], BF16)

            # ---- input DMAs: critical prefix inputs first, then w2, all on
            # the sync HWDGE queue so the small loads aren't round-robin
            # starved by the 3.2MB w2 stream on another queue ----
            mblk_sb = pre.tile([128, T], BF16)
            nc.sync.dma_start(mblk_sb[:], d_mblk[:])
            gram_sb = pre.tile([128, 4, 128], BF16)
            nc.sync.dma_start(gram_sb[:], d_gram[:])
            ew1_sb = pre.tile([128, 4, 128], BF16)
            nc.sync.dma_start(ew1_sb[:], d_ew1[:])
            sm_sb = pre.tile([128, 9], BF16)
            nc.sync.dma_start(sm_sb[:], d_sm[:])
            rv_sb = pre.tile([1, 256], F32R)
            nc.sync.dma_start(rv_sb[:], d_rv[:])
            rsum_sb = sm_sb          # cols 0:4
            ones_sb = sm_sb[:, 4:6]
            cu_sb = sm_sb[:, 6:9]
            onesr_sb = rv_sb[:, 0:128]
            nu_sb = rv_sb[:, 128:256]
            # w2 follows the inputs on the sync queue: FIFO gives the
            # small critical inputs full bandwidth (a parallel SWDGE w2
            # stream starves their tiny descriptors at engine round-robin),
            # and w2 drains before the first output block is ready
            for wc in range(NW2):
                wsl = slice(wc * W2CH, (wc + 1) * W2CH)
                nc.sync.dma_start(w2_sb[:, wsl], d_w2[:, wsl])
            # prime the ACT function tables (Square / Abs_reciprocal_sqrt)
            # so the 1.3us table loads happen during the input DMAs, not on
            # the LN critical path
            scratch_sb = pre.tile([128, 2], F32)
            nc.vector.memset(scratch_sb[:], 1.0)
            nc.scalar.square(scratch_sb[:, 0:1], scratch_sb[:, 1:2])
            nc.scalar.activation(scratch_sb[:, 0:1], scratch_sb[:, 1:2],
                                 ACTF.Abs_reciprocal_sqrt)
            if has_b2:
                b2r_sb = pre.tile([1, ES], F32R)
                nc.scalar.dma_start(b2r_sb[:], d_b2[:])

            # ---- prefix: stats + classifier input, no pooled tensor ----
            with tc.tile_pool(name="preps", bufs=1, space="PSUM") as prps:
                yT_ps = prps.tile([128, T], F32)
                wps_cm = tc.tile_pool(name="wps", bufs=1, space="PSUM")
                wps = wps_cm.__enter__()
                W_ps = wps.tile([128, T], F32)
                s2_ps = wps.tile([1, T], F32)
                mu_ps = wps.tile([1, T], F32)

                # W = G @ M and yT = (E w1g).T @ M and mu = rsum.T @ M
                # yT: only the first 256-slice per bank carries start=True so
                # the has_written bits stay set for the outer accumulate.
                for g in range(4):
                    gsl = slice(g * 256, (g + 1) * 256)
                    nc.tensor.matmul(out=W_ps[:, gsl], lhsT=gram_sb[:, g, :],
                                     rhs=mblk_sb[:, g * 256:(g + 1) * 256], start=True, stop=True,
                                     skip_group_check=True)
                    nc.tensor.matmul(out=yT_ps[:, gsl], lhsT=ew1_sb[:, g, :],
                                     rhs=mblk_sb[:, g * 256:(g + 1) * 256],
                                     start=(g % 2 == 0), stop=False,
                                     skip_group_check=True)
                    nc.tensor.matmul(out=mu_ps[:, gsl], lhsT=rsum_sb[:, g:g + 1],
                                     rhs=mblk_sb[:, g * 256:(g + 1) * 256], start=True, stop=True,
                                     skip_group_check=True)
                # WM = W (.) M  -> s2 = 1.T @ WM
                wm_sb = pre.tile([128, T], BF16)
                mu_sb = pre.tile([1, T], F32R)
                e2_sb = pre.tile([1, T], F32)
                musq_sb = pre.tile([1, T], F32)
                vare_sb = pre.tile([1, T], F32)
                rs_sb = pre.tile([1, T], F32R)
                for nch in range(2):
                    sl = slice(nch * 512, (nch + 1) * 512)
                    nc.vector.tensor_tensor(
                        wm_sb[:, sl], W_ps[:, sl],
                        mblk_sb[:, sl], op=AF.mult)
                    nc.tensor.matmul(out=s2_ps[:, sl], lhsT=ones_sb[:, 0:1],
                                     rhs=wm_sb[:, sl], start=True, stop=True,
                                     skip_group_check=True)
                # per-token scalars on [1, T], halves split across engines
                # (range deps let the h0 chain start after the g=0,1 MMs):
                #   var' = s2/D - mu^2 ; rs = rsqrt(var' + eps)  (eps folded
                #   into the rsqrt bias)
                h0 = slice(0, 512)
                h1 = slice(512, 1024)
                nc.scalar.copy(mu_sb[:, h0], mu_ps[:, h0])
                nc.scalar.square(musq_sb[:, h0], mu_sb[:, h0])
                nc.scalar.copy(mu_sb[:, h1], mu_ps[:, h1])
                # e2eps = s2/D + eps (one TS op), vare = e2eps - mu^2
                nc.vector.tensor_scalar(e2_sb[:, h0], s2_ps[:, h0], 1.0 / D,
                                        LN_EPS, op0=AF.mult, op1=AF.add)
                nc.vector.tensor_tensor(vare_sb[:, h0], e2_sb[:, h0],
                                        musq_sb[:, h0], op=AF.subtract)
                nc.vector.tensor_tensor(musq_sb[:, h1], mu_sb[:, h1],
                                        mu_sb[:, h1], op=AF.mult)
                nc.vector.tensor_scalar(e2_sb[:, h1], s2_ps[:, h1], 1.0 / D,
                                        LN_EPS, op0=AF.mult, op1=AF.add)
                nc.vector.tensor_tensor(vare_sb[:, h1], e2_sb[:, h1],
                                        musq_sb[:, h1], op=AF.subtract)
                # mean-subtraction as PE outer-product accumulate:
                #   yT_ps[r, t] += (-u[r]) * mu[t]
                for nch in range(2):
                    sl = slice(nch * 512, (nch + 1) * 512)
                    nc.tensor.matmul(out=yT_ps[:, sl], lhsT=nu_sb,
                                     rhs=mu_sb[:, sl], start=False, stop=True,
                                     skip_group_check=True)
                # PE warm-up: keep the tensor engine busy through the LN
                # stats phase so HAM reaches K=8/8 before the main stream
                # (results written to the already-consumed W_ps and thrown
                # away; the WAR dep on the wm reads keeps this safe)
                for wu in range(5):
                    nc.tensor.matmul(out=W_ps[:, 0:512], lhsT=gram_sb[:, wu % 4, :],
                                     rhs=mblk_sb[:, 0:512], start=True, stop=True,
                                     skip_group_check=True)
                wps_cm.__exit__(None, None, None)

                # broadcast rs across partitions; hT = rs (.) yT (+ c)
                with tc.tile_pool(name="bcps", bufs=1, space="PSUM") as bps:
                    rsb_ps = bps.tile([128, T], F32)
                    rsb_sb = pre.tile([128, T], F32)
                    if has_c:
                        t1_sb = pre.tile([128, T], BF16)
                    # both rsqrts (and their broadcasts) before the rsb
                    # copies: ACT is the serial tail of the prefix, and the
                    # main loop's first PSUM banks wait on its last consumer
                    for nch in range(2):
                        sl = slice(nch * 512, (nch + 1) * 512)
                        nc.scalar.activation(rs_sb[:, sl], vare_sb[:, sl],
                                             ACTF.Abs_reciprocal_sqrt)
                        nc.tensor.matmul(out=rsb_ps[:, sl], lhsT=onesr_sb,
                                         rhs=rs_sb[:, sl], start=True, stop=True,
                                         skip_group_check=True)
                    for nch in range(2):
                        sl = slice(nch * 512, (nch + 1) * 512)
                        nc.scalar.copy(rsb_sb[:, sl], rsb_ps[:, sl])
                        if has_c:
                            nc.vector.tensor_tensor(t1_sb[:, sl], yT_ps[:, sl],
                                                    rsb_sb[:, sl], op=AF.mult)
                            nc.vector.tensor_scalar(hT_sb[:, sl], t1_sb[:, sl],
                                                    cu_sb[:, 0:1], None, op0=AF.add)
                        else:
                            nc.vector.tensor_tensor(hT_sb[:, sl], yT_ps[:, sl],
                                                    rsb_sb[:, sl], op=AF.mult)

            # ---- main loop: 100 minigroups of 2 x (N=500) chunks ----
            # PSUM pool bufs=4 x [128, 2, 512] (2 banks each).  Minigroup
            # mg -> buffer mg%4; DVE evacuates buffers 0-1, ACT buffers 2-3.
            # Four col-adjacent minigroups (one 4000-col block, same tt)
            # share an o2 tile and one 1MB DMA on sync.  Tail minigroups
            # (cols 12000:12500 across tts) pair up into 128KB SWDGE DMAs.
            mgs = []
            for tt in range(TT):
                for g in range(12):
                    mgs.append(("big", tt, g, [(tt, g * 1000 + k * 500)
                                               for k in range(2)]))
                if tt == 3:
                    mgs.append(("tail", 0, 0, [(0, 12000), (1, 12000)]))
                    mgs.append(("tail", 0, 1, [(2, 12000), (3, 12000)]))
                if tt == 7:
                    mgs.append(("tail", 1, 0, [(4, 12000), (5, 12000)]))
                    mgs.append(("tail", 1, 1, [(6, 12000), (7, 12000)]))

            with tc.tile_pool(name="mainps", bufs=4, space="PSUM") as mps, \
                 tc.tile_pool(name="outp", bufs=8) as op, \
                 tc.tile_pool(name="outtp", bufs=2) as otp:
                o2p = None
                o2t = None
                for mg, (kind, tt, g, chunks) in enumerate(mgs):
                    ps = mps.tile([128, 2, 512], F32, tag="ps")
                    for k, (ctt, cs) in enumerate(chunks):
                        nc.tensor.matmul(out=ps[:, k, 0:ECH],
                                         lhsT=hT_sb[:, ctt * 128:(ctt + 1) * 128],
                                         rhs=w2_sb[:, cs:cs + ECH],
                                         start=True, stop=not has_b2)
                        if has_b2:
                            nc.tensor.matmul(out=ps[:, k, 0:ECH],
                                             lhsT=onesr_sb,
                                             rhs=b2r_sb[:, cs:cs + ECH],
                                             start=False, stop=True)
                    # evac engine follows the PSUM buffer (mg%4):
                    # DVE owns buffers 0-1 (banks 0-3), ACT 2-3 (banks 4-7)
                    evac = (nc.vector.tensor_copy if mg % 4 in (0, 1)
                            else nc.scalar.copy)
                    if kind == "big":
                        if g % 2 == 0:
                            o2p = op.tile([128, 2, 2, ECH], BF16, tag="o2",
                                          name=f"o2_{mg}")
                        evac(o2p[:, g % 2, :, :], ps[:, :, 0:ECH])
                        if g % 2 == 1:
                            blk = 6 * tt + g // 2
                            nc.sync.dma_start(
                                d_outb[blk * 128:(blk + 1) * 128, :],
                                o2p[:])
                            o2p = None
                    else:
                        if g == 0:
                            o2t = otp.tile([128, 2, 2, ECH], BF16, tag="o2t",
                                           name=f"o2t_{mg}")
                        evac(o2t[:, g, :, :], ps[:, :, 0:ECH])
                        if g == 1:
                            for q in range(4):
                                qq = tt * 4 + q
                                nc.gpsimd.dma_start(
                                    d_outt[qq * 128:(qq + 1) * 128, :],
                                    o2t[:, q // 2, q % 2, :])
                            o2t = None

    nc.finalize()
    return nc


_NC_CACHE = {}


def _get_nc(has_b2: bool, has_c: bool):
    key = (has_b2, has_c)
    if key not in _NC_CACHE:
        _NC_CACHE[key] = build_nc(has_b2, has_c)
    return _NC_CACHE[key]


def prep_core_inputs(inputs):
    """Host-side sharding/layout prep. Returns (shared_map, per_core_w2, per_core_b2, has_b2, has_c)."""
    ids = np.asarray(inputs["input_ids"]).astype(np.int32)[:, :L]      # [16, 32]
    pos = np.asarray(inputs["entity_position_ids"]).astype(np.int32)   # [16, 64, 32]
    am = np.asarray(inputs["entity_attention_mask"]).astype(np.int32)  # [16, 64]
    embed = np.ascontiguousarray(np.asarray(inputs["embed"], dtype=np.float32))
    gamma = np.asarray(inputs["ln_gamma"], dtype=np.float32)
    beta = np.asarray(inputs["ln_beta"], dtype=np.float32)
    w1 = np.ascontiguousarray(np.asarray(inputs["w1"], dtype=np.float32))
    w2 = np.asarray(inputs["w2"], dtype=np.float32)
    b2 = np.asarray(inputs["b2"], dtype=np.float32)

    # E_g[p=32b+x, g, :] = embed[ids[4g+b, x]]  (batch 4g+b on k-block b)
    emb_idx = ids.reshape(4, 4, L).transpose(1, 2, 0).reshape(128, 4)
    emb_g = embed[emb_idx]                                  # [128, 4, D]
    rsum = np.ascontiguousarray(
        emb_g.sum(axis=2, dtype=np.float64).astype(np.float32) / D)   # [128, 4]

    # block-diagonal mask, scaled by 1/L:
    #   mblk[32k+x, g, 64k+j] = m[4g+k, j, x] / L
    mask = (((pos != -1) & (am[:, :, None] != 0)).astype(np.float32) / L)  # [b,j,x]
    mblk = np.zeros((128, 4, 256), np.float32)
    for g in range(4):
        for k in range(4):
            mblk[32 * k:32 * (k + 1), g, 64 * k:64 * (k + 1)] = \
                mask[4 * g + k].T
    mblk = np.ascontiguousarray(mblk.reshape(128, 4 * 256))

    # host-folded small matrices (mask-independent):
    #   gram[:, g, :] = E_g @ E_g.T ; ew1[:, g, :] = E_g @ (gamma (.) w1)
    w1g = gamma[:, None] * w1                               # [D, R]
    eg64 = emb_g.astype(np.float64)
    gram = np.einsum('pgd,qgd->pgq', eg64, eg64).astype(np.float32)
    ew1 = np.einsum('pgd,dr->pgr', eg64, w1g.astype(np.float64)).astype(np.float32)
    u = gamma @ w1                                          # [R]
    cvec = beta @ w1                                        # [R]
    cu = np.ascontiguousarray(
        np.stack([cvec, u, -u], axis=-1))                   # [R, 3]

    bf = ml_dtypes.bfloat16
    sm = np.concatenate([rsum, np.ones((128, 2), np.float32), cu], axis=1)
    rv = np.concatenate([np.ones(128, np.float32), -u]).reshape(1, 256)
    shared = {
        "mblk": mblk.astype(bf),
        "gram": np.ascontiguousarray(gram.reshape(128, 4 * 128)).astype(bf),
        "ew1": np.ascontiguousarray(ew1.reshape(128, 4 * 128)).astype(bf),
        "smalls": np.ascontiguousarray(sm).astype(bf),
        "rowv": np.ascontiguousarray(rv),
    }
    w2s = [np.ascontiguousarray(w2[:, c * ES:(c + 1) * ES]).astype(ml_dtypes.bfloat16)
           for c in range(N_CORES)]
    b2s = [np.ascontiguousarray(b2[c * ES:(c + 1) * ES].reshape(1, ES))
           for c in range(N_CORES)]
    has_b2 = bool(np.any(b2 != 0.0))
    has_c = bool(np.any(cvec != 0.0))
    return shared, w2s, b2s, has_b2, has_c


def kernel(**inputs) -> np.ndarray:
    shared, w2s, b2s, has_b2, has_c = prep_core_inputs(inputs)
    nc = _get_nc(has_b2, has_c)
    in_maps = [dict(shared, w2s=w2s[c], b2s=b2s[c]) for c in range(N_CORES)]
    res = run_bass_kernel_spmd(nc, in_maps, list(range(N_CORES)))
    full = np.empty((T, E), np.float32)
    for c in range(N_CORES):
        co = c * ES
        outb = np.asarray(res.results[c]["outb"]).astype(np.float32)
        outt = np.asarray(res.results[c]["outt"]).astype(np.float32)
        # outb[(3*tt+j)*128 + p, :] -> full[tt*128 + p, co + 4000*j : +4000]
        ob = outb.reshape(TT, 6, 128, 2000).transpose(0, 2, 1, 3).reshape(T, 12000)
        full[:, co:co + 12000] = ob
        full[:, co + 12000:co + 12500] = outt
    return np.ascontiguousarray(full.reshape(B, M, E))
